# revision 8
# baseline (speedup 1.0000x reference)
"""Trainium2 Bass kernel for nn_AttentionContextEncoder (gnn_message_passing).

reference:
  ents = ctx.T.reshape(B, 7, 4)
  prop_emb = relu(ents @ w_prop + b_prop)                      # [B,7,128]
  diffs[b,i,j,:] = ents[b,i,:] - ents[b,j,:]
  dist = sqrt(diffs[...,0]^2 + diffs[...,1]^2)
  rel = relu(concat([diffs, dist]) @ w_rel + b_rel)            # [B,7,7,128]
  rel_emb = sum_{j != i} rel[:, i, j, :]                       # [B,7,128]
  out = concat([prop_emb, rel_emb], -1)                        # [B,7,256]

v6 design (data-parallel over 8 cores, B=2048/core):
- Math as v2..v5: host G (diff combos), R (sq-dist reduce), wp/wm/wq
  weight images with bias folded via a ones contraction row; one K=6
  matmul per directed pair (512-col chunks).
- Measured: PE is pinned at 1.2 GHz (no HAM warm-up) -> 427ns per
  512-col MM, but 4-strip tile_position rotation overlaps MMs almost
  perfectly.  Engine floor is the f32 PSUM first-read: every pair tile
  must cross DVE/ACT once (~62us combined).  Prior versions lost 40+us
  to cross-engine dependency stalls (GpSimd combines, chain in1 deps,
  shared PSUM ring).
- v6: NO on-chip combines.  Each target's 6 pairs: 3 -> ACT relu into
  individual r tiles (DMA'd out), 3 -> DVE relu-accumulate chain into
  acc (ts for the first, stt for the rest; DMA'd out).  The HOST sums
  rel = acc + r0 + r1 + r2.  GpSimd does zero compute: it is a DMA
  issue queue (plus the diff-squares).  D-pairs stream as 4-pair
  windows of [128,512] quarter slots (4-strip rotation); A-pairs as
  [128,1024] halves, chunk-interleaved 1:1 with the D stream.
- Output bf16 [5,7,128,B] per core (prop, acc, r0, r1, r2); host sums,
  converts to f32, transposes.
"""
import numpy as np
import ml_dtypes
from contextlib import ExitStack

import concourse.bass as bass
import concourse.bacc as bacc
import concourse.mybir as mybir
import concourse.tile as tile
from concourse.bass_utils import run_bass_kernel_spmd

F32 = mybir.dt.float32
BF16 = mybir.dt.bfloat16
AF = mybir.ActivationFunctionType
ALU = mybir.AluOpType

NUM_ENT = 7
DIM_ENT = 4
H = 128
B_TOTAL = 16384
N_CORES = 8
B = B_TOTAL // N_CORES          # 2048 per core
HB = B // 2                     # 1024
QB = B // 4                     # 512  (one PSUM bank)

_CLS = [[] for _ in range(7)]
for i in range(NUM_ENT):
    for j in range(i + 1, NUM_ENT):
        _CLS[(i + j) % 7].append((i, j))
_STRIP_PAIRS = [_CLS[0] + _CLS[1], _CLS[2] + _CLS[3], _CLS[4] + _CLS[5], _CLS[6]]
PAIRS = [p for sp in _STRIP_PAIRS for p in sp]
STRIP_NP = [len(sp) for sp in _STRIP_PAIRS]          # [6, 6, 6, 3]
STRIP_START = [0, 6, 12, 18]
PAIR_SG = {}
for s in range(4):
    for g in range(STRIP_NP[s]):
        PAIR_SG[STRIP_START[s] + g] = (s, g)
PAIR_IDX = {PAIRS[k]: k for k in range(21)}

PROP_SG = {t: (t % 4, t // 4) for t in range(NUM_ENT)}
PROP_DVE_H1 = {2, 3, 4, 5, 6}    # prop h1 drained by DVE for these targets


def _ordered_pairs(t):
    by_strip = [[] for _ in range(4)]
    for j in range(NUM_ENT):
        if j == t:
            continue
        a, b_ = (t, j) if t < j else (j, t)
        s, _ = PAIR_SG[PAIR_IDX[(a, b_)]]
        by_strip[s].append(j)
    order = []
    r = 0
    while len(order) < 6:
        for s in range(4):
            if len(by_strip[s]) > r:
                order.append(by_strip[s][r])
        r += 1
    return order


def build_constants(w_prop, b_prop, w_rel, b_rel):
    bf = ml_dtypes.bfloat16
    G = np.zeros((NUM_ENT * DIM_ENT, 84), np.float32)
    for k, (i, j) in enumerate(PAIRS):
        for c in range(DIM_ENT):
            G[4 * i + c, 21 * c + k] = 1.0
            G[4 * j + c, 21 * c + k] = -1.0
    R = np.zeros((42, 21), np.float32)
    for k in range(21):
        R[k, k] = 1.0
        R[21 + k, k] = 1.0
    wp = np.zeros((H, H), np.float32)
    wm = np.zeros((H, H), np.float32)
    wq = np.zeros((H, H), np.float32)
    for s in range(4):
        r0 = 32 * s
        wp[r0:r0 + 4, :] = w_rel[0:4]
        wp[r0 + 4, :] = w_rel[4]
        wp[r0 + 5, :] = b_rel
        wm[r0:r0 + 4, :] = -w_rel[0:4]
        wm[r0 + 4, :] = w_rel[4]
        wm[r0 + 5, :] = b_rel
        wq[r0:r0 + 4, :] = w_prop
        wq[r0 + 4, :] = b_prop
    ones6 = np.ones((6, B), np.float32)
    return {
        "gmat": G.astype(bf), "rmat": R.astype(bf),
        "wpimg": wp.astype(bf), "wmimg": wm.astype(bf), "wqimg": wq.astype(bf),
        "onesb": ones6.astype(bf),
    }


def build():
    nc = bacc.Bacc("TRN2", target_bir_lowering=False, debug=False,
                   num_devices=N_CORES)
    ctxb_d = nc.dram_tensor("ctxb", [NUM_ENT * DIM_ENT, B], BF16,
                            kind="ExternalInput").ap()
    gmat_d = nc.dram_tensor("gmat", [NUM_ENT * DIM_ENT, 84], BF16,
                            kind="ExternalInput").ap()
    rmat_d = nc.dram_tensor("rmat", [42, 21], BF16, kind="ExternalInput").ap()
    wp_d = nc.dram_tensor("wpimg", [H, H], BF16, kind="ExternalInput").ap()
    wm_d = nc.dram_tensor("wmimg", [H, H], BF16, kind="ExternalInput").ap()
    wq_d = nc.dram_tensor("wqimg", [H, H], BF16, kind="ExternalInput").ap()
    ones_d = nc.dram_tensor("onesb", [6, B], BF16, kind="ExternalInput").ap()
    # out[0]=prop, out[1]=acc(3 DVE pairs), out[2..4]=r0..r2 (ACT pairs)
    out_d = nc.dram_tensor("out", [5, NUM_ENT, H, B], BF16,
                           kind="ExternalOutput").ap()

    with tile.TileContext(nc) as tc, ExitStack() as ctx:
        stat = ctx.enter_context(tc.tile_pool(name="stat", bufs=1))
        rp = ctx.enter_context(tc.tile_pool(name="rp", bufs=6))
        accp = ctx.enter_context(tc.tile_pool(name="accp", bufs=3))
        poutp = ctx.enter_context(tc.tile_pool(name="poutp", bufs=4))

        pslq = ctx.enter_context(tc.tile_pool(name="pslq", bufs=4,
                                              space="PSUM"))
        pslh = ctx.enter_context(tc.tile_pool(name="pslh", bufs=2,
                                              space="PSUM"))

        # ---------- inputs ----------
        ctxb = stat.tile([NUM_ENT * DIM_ENT, B], BF16)
        nc.sync.dma_start(ctxb[:], ctxb_d[:])
        gm = stat.tile([NUM_ENT * DIM_ENT, 84], BF16)
        nc.sync.dma_start(gm[:], gmat_d[:])
        wqimg = stat.tile([H, H], BF16)
        nc.sync.dma_start(wqimg[:], wq_d[:])
        onesb = stat.tile([6, B], BF16)
        nc.gpsimd.dma_start(onesb[:], ones_d[:])
        rm = stat.tile([42, 21], BF16)
        nc.gpsimd.dma_start(rm[:], rmat_d[:])
        wpimg = stat.tile([H, H], BF16)
        nc.gpsimd.dma_start(wpimg[:], wp_d[:])
        wmimg = stat.tile([H, H], BF16)
        nc.gpsimd.dma_start(wmimg[:], wm_d[:])

        prop6 = stat.tile([H, 2, B], BF16)
        for t in range(NUM_ENT):
            ps_, pg_ = PROP_SG[t]
            eng = nc.sync if t % 2 == 0 else nc.gpsimd
            eng.dma_start(prop6[32 * ps_:32 * ps_ + 4, pg_, :],
                          ctxb_d[4 * t:4 * t + 4, :])
        for s in range(4):
            nc.gpsimd.dma_start(prop6[32 * s + 4:32 * s + 5, 0:2, :],
                                onesb[0:2, :])

        # ---------- prep: G -> cmpb ----------
        cmpb = stat.tile([84, B], BF16)
        for h in range(2):
            cslot = pslh.tile([84, HB], F32, tag="hs", name="cslot")
            for c in range(2):
                nc.tensor.matmul(cslot[:, 512 * c:512 * c + 512],
                                 gm[:, :],
                                 ctxb[:, HB * h + 512 * c:HB * h + 512 * c + 512],
                                 start=True, stop=True, tile_position=(0, 0))
            nc.scalar.copy(cmpb[:, HB * h:HB * h + HB], cslot[:])

        def emit_prop(t):
            ps_, pg_ = PROP_SG[t]
            pout = poutp.tile([H, B], BF16, tag="pout", name="pout")
            for h in range(2):
                slot = pslh.tile([H, HB], F32, tag="hs", name="pslot")
                for c in range(2):
                    nc.tensor.matmul(
                        slot[:, 512 * c:512 * c + 512],
                        wqimg[32 * ps_:32 * ps_ + 5, :],
                        prop6[32 * ps_:32 * ps_ + 5, pg_,
                              HB * h + 512 * c:HB * h + 512 * c + 512],
                        start=True, stop=True, tile_position=(32 * ps_, 0))
                if h == 1 and t in PROP_DVE_H1:
                    nc.vector.tensor_single_scalar(
                        pout[:, HB * h:HB * h + HB], slot[:], 0.0, op=ALU.max)
                else:
                    nc.scalar.activation(pout[:, HB * h:HB * h + HB],
                                         slot[:], AF.Relu)
            nc.sync.dma_start(out_d[0, t, :, :], pout[:])

        emit_prop(0)
        emit_prop(1)

        # squares on GpSimd (frees DVE); R matmuls; sqrt on ACT
        sq = stat.tile([42, B], BF16)
        nc.gpsimd.tensor_mul(sq[0:42, :], cmpb[0:42, :], cmpb[0:42, :])
        distb = stat.tile([21, B], BF16)
        for h in range(2):
            dslot = pslh.tile([21, HB], F32, tag="hs", name="dslot")
            for c in range(2):
                nc.tensor.matmul(dslot[:, 512 * c:512 * c + 512],
                                 rm[0:42, :],
                                 sq[0:42, HB * h + 512 * c:HB * h + 512 * c + 512],
                                 start=True, stop=True, tile_position=(0, 0))
            nc.scalar.activation(distb[:, HB * h:HB * h + HB], dslot[:],
                                 AF.Sqrt)

        # ---------- rhs6 staging (sync + gpsimd issue queues) ----------
        rhs6 = stat.tile([H, 6, B], BF16)
        qs = [nc.sync, nc.gpsimd]
        qi = 0
        for s in range(4):
            k0, np_ = STRIP_START[s], STRIP_NP[s]
            for c in range(DIM_ENT):
                qs[qi % 2].dma_start(rhs6[32 * s + c:32 * s + c + 1, 0:np_, :],
                                     cmpb[21 * c + k0:21 * c + k0 + np_, :])
                qi += 1
            qs[qi % 2].dma_start(rhs6[32 * s + 4:32 * s + 5, 0:np_, :],
                                 distb[k0:k0 + np_, :])
            qi += 1
            qs[qi % 2].dma_start(rhs6[32 * s + 5:32 * s + 6, 0:6, :],
                                 onesb[0:6, :])
            qi += 1

        # ---------- main: D windows + A chunk stream ----------
        def pair_mm(t, j, slot_ap, col0, ncols):
            a, b_ = (t, j) if t < j else (j, t)
            s, g = PAIR_SG[PAIR_IDX[(a, b_)]]
            img = wpimg if t < j else wmimg
            nc.tensor.matmul(
                slot_ap, img[32 * s:32 * s + 6, :],
                rhs6[32 * s:32 * s + 6, g, col0:col0 + ncols],
                start=True, stop=True, tile_position=(32 * s, 0))

        A_items = []                     # (t, j, widx 0..2)
        D_items = []                     # (t, j, didx 0..2)
        tstate = {}
        for t in range(NUM_ENT):
            order = _ordered_pairs(t)
            tstate[t] = {
                "acc": accp.tile([H, B], BF16, tag="acc", name="acc"),
                "r": [None, None, None],
            }
            for w in range(3):
                A_items.append((t, order[2 * w], w))
                D_items.append((t, order[2 * w + 1], w))

        astate = {"idx": 0, "hs": None, "r": None}

        def emit_a_chunk():
            i = astate["idx"]
            if i >= 84:
                return
            astate["idx"] = i + 1
            item, c = i // 4, i % 4
            t, j, widx = A_items[item]
            half, cih = c // 2, c % 2
            if c == 0:
                astate["r"] = rp.tile([H, B], BF16, tag="r", name="r")
                tstate[t]["r"][widx] = astate["r"]
            if cih == 0:
                astate["hs"] = pslh.tile([H, HB], F32, tag="hs", name="ahs")
            pair_mm(t, j, astate["hs"][:, 512 * cih:512 * cih + 512],
                    HB * half + 512 * cih, 512)
            if cih == 1:
                nc.scalar.activation(
                    astate["r"][:, HB * half:HB * half + HB],
                    astate["hs"][:], AF.Relu)
                if half == 1:
                    nc.sync.dma_start(out_d[2 + widx, t, :, :],
                                      astate["r"][:])

        nwin = (len(D_items) + 3) // 4
        for k in range(nwin):
            dwin = D_items[4 * k:4 * k + 4]
            for qc in range(4):
                qslots = []
                for (t, j, didx) in dwin:
                    qslot = pslq.tile([H, QB], F32, tag="qs", name="qslot")
                    qslots.append(qslot)
                    pair_mm(t, j, qslot[:, :], QB * qc, QB)
                    emit_a_chunk()
                for (t, j, didx), qslot in zip(dwin, qslots):
                    acc = tstate[t]["acc"]
                    dst = acc[:, QB * qc:QB * qc + QB]
                    if didx == 0:
                        nc.vector.tensor_single_scalar(dst, qslot[:], 0.0,
                                                       op=ALU.max)
                    else:
                        nc.vector.scalar_tensor_tensor(
                            dst, qslot[:], 0.0, dst, op0=ALU.max, op1=ALU.add)
                    if didx == 2 and qc == 3:
                        nc.gpsimd.dma_start(out_d[1, t, :, :], acc[:])
            if k < 5:
                emit_prop(k + 2)
        # drain any remaining A chunks (shouldn't be needed: 84==84)
        while astate["idx"] < 84:
            emit_a_chunk()

    nc.compile()
    return nc


_NC_CACHE = None


def _get_nc():
    global _NC_CACHE
    if _NC_CACHE is None:
        _NC_CACHE = build()
    return _NC_CACHE


def run(ctx, w_prop, b_prop, w_rel, b_rel, trace=False):
    bf = ml_dtypes.bfloat16
    ctx = np.asarray(ctx, dtype=np.float32)
    nc = _get_nc()
    shared = build_constants(np.asarray(w_prop, np.float32),
                             np.asarray(b_prop, np.float32),
                             np.asarray(w_rel, np.float32),
                             np.asarray(b_rel, np.float32))
    in_maps = []
    for c in range(N_CORES):
        m = dict(shared)
        m["ctxb"] = np.ascontiguousarray(ctx[:, c * B:(c + 1) * B]).astype(bf)
        in_maps.append(m)
    res = run_bass_kernel_spmd(nc, in_maps, core_ids=list(range(N_CORES)),
                               trace=trace)
    shards = [np.asarray(res.results[c]["out"]).astype(np.float32)
              for c in range(N_CORES)]
    full = np.concatenate(shards, axis=3)                   # [5,7,128,16384]
    prop = full[0]
    rel = full[1] + full[2] + full[3] + full[4]
    comb = np.stack([prop, rel], axis=2)                    # [7,128,2,16384]
    out = np.transpose(comb, (3, 0, 2, 1)).reshape(B_TOTAL, NUM_ENT, 2 * H)
    return np.ascontiguousarray(out), res


def kernel(ctx, w_prop, b_prop, w_rel, b_rel):
    return run(ctx, w_prop, b_prop, w_rel, b_rel)[0]


# revision 9
# speedup vs baseline: 1.1166x; 1.1166x over previous
"""Trainium2 Bass kernel for nn_AttentionContextEncoder (gnn_message_passing).

reference:
  ents = ctx.T.reshape(B, 7, 4)
  prop_emb = relu(ents @ w_prop + b_prop)                      # [B,7,128]
  diffs[b,i,j,:] = ents[b,i,:] - ents[b,j,:]
  dist = sqrt(diffs[...,0]^2 + diffs[...,1]^2)
  rel = relu(concat([diffs, dist]) @ w_rel + b_rel)            # [B,7,7,128]
  rel_emb = sum_{j != i} rel[:, i, j, :]                       # [B,7,128]
  out = concat([prop_emb, rel_emb], -1)                        # [B,7,256]

v7 design (data-parallel over 8 cores, B=2048/core):
- Math as before: host G/R/weight images, K=6 matmul per directed pair.
- PE pinned at 1.2 GHz; 4-strip tile_position rotation overlaps MMs.
  Engine floor = f32 PSUM first-reads through DVE+ACT (~62us).
- No on-chip combines: 3 pairs/target -> ACT relu r-tiles (DMA'd out),
  3 pairs/target -> DVE relu-accumulate chains into acc (DMA'd out);
  host sums rel = acc + r0 + r1 + r2.  GpSimd only squares + DMA issue.
- Emission in phase blocks to avoid in-order PE-queue coupling: each
  D-round is 4 quarter MMs from 4 different-strip pairs (clean burst),
  followed by ~2 A-half blocks (2 chunks + relu).  Bootstrap: props
  0-3 front-loaded, staging ordered ones->diffs->dist, sq on DVE.
- Output bf16 [5,7,128,B]; host sums/transposes.
"""
import numpy as np
import ml_dtypes
from contextlib import ExitStack

import concourse.bass as bass
import concourse.bacc as bacc
import concourse.mybir as mybir
import concourse.tile as tile
from concourse.bass_utils import run_bass_kernel_spmd

F32 = mybir.dt.float32
BF16 = mybir.dt.bfloat16
AF = mybir.ActivationFunctionType
ALU = mybir.AluOpType

NUM_ENT = 7
DIM_ENT = 4
H = 128
B_TOTAL = 16384
N_CORES = 8
B = B_TOTAL // N_CORES          # 2048 per core
HB = B // 2                     # 1024
QB = B // 4                     # 512  (one PSUM bank)

_CLS = [[] for _ in range(7)]
for i in range(NUM_ENT):
    for j in range(i + 1, NUM_ENT):
        _CLS[(i + j) % 7].append((i, j))
_STRIP_PAIRS = [_CLS[0] + _CLS[1], _CLS[2] + _CLS[3], _CLS[4] + _CLS[5], _CLS[6]]
PAIRS = [p for sp in _STRIP_PAIRS for p in sp]
STRIP_NP = [len(sp) for sp in _STRIP_PAIRS]          # [6, 6, 6, 3]
STRIP_START = [0, 6, 12, 18]
PAIR_SG = {}
for s in range(4):
    for g in range(STRIP_NP[s]):
        PAIR_SG[STRIP_START[s] + g] = (s, g)
PAIR_IDX = {PAIRS[k]: k for k in range(21)}

PROP_SG = {t: (t % 4, t // 4) for t in range(NUM_ENT)}
PROP_DVE_H1 = {2, 3, 4, 5, 6}


def _ordered_pairs(t):
    by_strip = [[] for _ in range(4)]
    for j in range(NUM_ENT):
        if j == t:
            continue
        a, b_ = (t, j) if t < j else (j, t)
        s, _ = PAIR_SG[PAIR_IDX[(a, b_)]]
        by_strip[s].append(j)
    order = []
    r = 0
    while len(order) < 6:
        for s in range(4):
            if len(by_strip[s]) > r:
                order.append(by_strip[s][r])
        r += 1
    return order


def build_constants(w_prop, b_prop, w_rel, b_rel):
    bf = ml_dtypes.bfloat16
    G = np.zeros((NUM_ENT * DIM_ENT, 84), np.float32)
    for k, (i, j) in enumerate(PAIRS):
        for c in range(DIM_ENT):
            G[4 * i + c, 21 * c + k] = 1.0
            G[4 * j + c, 21 * c + k] = -1.0
    R = np.zeros((42, 21), np.float32)
    for k in range(21):
        R[k, k] = 1.0
        R[21 + k, k] = 1.0
    wp = np.zeros((H, H), np.float32)
    wm = np.zeros((H, H), np.float32)
    wq = np.zeros((H, H), np.float32)
    for s in range(4):
        r0 = 32 * s
        wp[r0:r0 + 4, :] = w_rel[0:4]
        wp[r0 + 4, :] = w_rel[4]
        wp[r0 + 5, :] = b_rel
        wm[r0:r0 + 4, :] = -w_rel[0:4]
        wm[r0 + 4, :] = w_rel[4]
        wm[r0 + 5, :] = b_rel
        wq[r0:r0 + 4, :] = w_prop
        wq[r0 + 4, :] = b_prop
    ones6 = np.ones((6, B), np.float32)
    return {
        "gmat": G.astype(bf), "rmat": R.astype(bf),
        "wpimg": wp.astype(bf), "wmimg": wm.astype(bf), "wqimg": wq.astype(bf),
        "onesb": ones6.astype(bf),
    }


def build():
    nc = bacc.Bacc("TRN2", target_bir_lowering=False, debug=False,
                   num_devices=N_CORES)
    ctxb_d = nc.dram_tensor("ctxb", [NUM_ENT * DIM_ENT, B], BF16,
                            kind="ExternalInput").ap()
    gmat_d = nc.dram_tensor("gmat", [NUM_ENT * DIM_ENT, 84], BF16,
                            kind="ExternalInput").ap()
    rmat_d = nc.dram_tensor("rmat", [42, 21], BF16, kind="ExternalInput").ap()
    wp_d = nc.dram_tensor("wpimg", [H, H], BF16, kind="ExternalInput").ap()
    wm_d = nc.dram_tensor("wmimg", [H, H], BF16, kind="ExternalInput").ap()
    wq_d = nc.dram_tensor("wqimg", [H, H], BF16, kind="ExternalInput").ap()
    ones_d = nc.dram_tensor("onesb", [6, B], BF16, kind="ExternalInput").ap()
    out_d = nc.dram_tensor("out", [5, NUM_ENT, H, B], BF16,
                           kind="ExternalOutput").ap()

    with tile.TileContext(nc) as tc, ExitStack() as ctx:
        stat = ctx.enter_context(tc.tile_pool(name="stat", bufs=1))
        rp = ctx.enter_context(tc.tile_pool(name="rp", bufs=6))
        accp = ctx.enter_context(tc.tile_pool(name="accp", bufs=3))
        poutp = ctx.enter_context(tc.tile_pool(name="poutp", bufs=4))

        pslq = ctx.enter_context(tc.tile_pool(name="pslq", bufs=4,
                                              space="PSUM"))
        pslh = ctx.enter_context(tc.tile_pool(name="pslh", bufs=2,
                                              space="PSUM"))

        # ---------- inputs ----------
        ctxb = stat.tile([NUM_ENT * DIM_ENT, B], BF16)
        nc.sync.dma_start(ctxb[:], ctxb_d[:])
        gm = stat.tile([NUM_ENT * DIM_ENT, 84], BF16)
        nc.sync.dma_start(gm[:], gmat_d[:])
        wqimg = stat.tile([H, H], BF16)
        nc.sync.dma_start(wqimg[:], wq_d[:])
        onesb = stat.tile([6, B], BF16)
        nc.gpsimd.dma_start(onesb[:], ones_d[:])
        rm = stat.tile([42, 21], BF16)
        nc.gpsimd.dma_start(rm[:], rmat_d[:])
        wpimg = stat.tile([H, H], BF16)
        nc.gpsimd.dma_start(wpimg[:], wp_d[:])
        wmimg = stat.tile([H, H], BF16)
        nc.gpsimd.dma_start(wmimg[:], wm_d[:])

        prop6 = stat.tile([H, 2, B], BF16)
        for t in range(NUM_ENT):
            ps_, pg_ = PROP_SG[t]
            nc.sync.dma_start(prop6[32 * ps_:32 * ps_ + 4, pg_, :],
                              ctxb_d[4 * t:4 * t + 4, :])
        for s in range(4):
            nc.sync.dma_start(prop6[32 * s + 4:32 * s + 5, 0:2, :],
                              onesb[0:2, :])

        # rhs6 ones rows depend only on onesb: stage first
        rhs6 = stat.tile([H, 6, B], BF16)
        for s in range(4):
            nc.gpsimd.dma_start(rhs6[32 * s + 5:32 * s + 6, 0:6, :],
                                onesb[0:6, :])

        # ---------- prep: G -> cmpb ----------
        cmpb = stat.tile([84, B], BF16)
        for h in range(2):
            cslot = pslh.tile([84, HB], F32, tag="hs", name="cslot")
            for c in range(2):
                nc.tensor.matmul(cslot[:, 512 * c:512 * c + 512],
                                 gm[:, :],
                                 ctxb[:, HB * h + 512 * c:HB * h + 512 * c + 512],
                                 start=True, stop=True, tile_position=(0, 0))
            nc.scalar.copy(cmpb[:, HB * h:HB * h + HB], cslot[:])

        def emit_prop(t):
            ps_, pg_ = PROP_SG[t]
            pout = poutp.tile([H, B], BF16, tag="pout", name="pout")
            for h in range(2):
                slot = pslh.tile([H, HB], F32, tag="hs", name="pslot")
                for c in range(2):
                    nc.tensor.matmul(
                        slot[:, 512 * c:512 * c + 512],
                        wqimg[32 * ps_:32 * ps_ + 5, :],
                        prop6[32 * ps_:32 * ps_ + 5, pg_,
                              HB * h + 512 * c:HB * h + 512 * c + 512],
                        start=True, stop=True, tile_position=(32 * ps_, 0))
                if h == 1 and t in PROP_DVE_H1:
                    nc.vector.tensor_single_scalar(
                        pout[:, HB * h:HB * h + HB], slot[:], 0.0, op=ALU.max)
                else:
                    nc.scalar.activation(pout[:, HB * h:HB * h + HB],
                                         slot[:], AF.Relu)
            nc.sync.dma_start(out_d[0, t, :, :], pout[:])

        # sq on DVE (short critical path), then props to cover staging
        sq = stat.tile([42, B], BF16)
        nc.vector.tensor_mul(sq[0:42, :], cmpb[0:42, :], cmpb[0:42, :])

        # diff rows staging (needs only cmpb) — before R/sqrt
        qs = [nc.sync, nc.gpsimd]
        qi = 0
        for s in range(4):
            k0, np_ = STRIP_START[s], STRIP_NP[s]
            for c in range(DIM_ENT):
                qs[qi % 2].dma_start(rhs6[32 * s + c:32 * s + c + 1, 0:np_, :],
                                     cmpb[21 * c + k0:21 * c + k0 + np_, :])
                qi += 1

        emit_prop(0)
        emit_prop(1)

        distb = stat.tile([21, B], BF16)
        for h in range(2):
            dslot = pslh.tile([21, HB], F32, tag="hs", name="dslot")
            for c in range(2):
                nc.tensor.matmul(dslot[:, 512 * c:512 * c + 512],
                                 rm[0:42, :],
                                 sq[0:42, HB * h + 512 * c:HB * h + 512 * c + 512],
                                 start=True, stop=True, tile_position=(0, 0))
            nc.scalar.activation(distb[:, HB * h:HB * h + HB], dslot[:],
                                 AF.Sqrt)
        for s in range(4):
            k0, np_ = STRIP_START[s], STRIP_NP[s]
            qs[qi % 2].dma_start(rhs6[32 * s + 4:32 * s + 5, 0:np_, :],
                                 distb[k0:k0 + np_, :])
            qi += 1

        emit_prop(2)
        emit_prop(3)

        # ---------- main ----------
        def pair_mm(t, j, slot_ap, col0, ncols):
            a, b_ = (t, j) if t < j else (j, t)
            s, g = PAIR_SG[PAIR_IDX[(a, b_)]]
            img = wpimg if t < j else wmimg
            nc.tensor.matmul(
                slot_ap, img[32 * s:32 * s + 6, :],
                rhs6[32 * s:32 * s + 6, g, col0:col0 + ncols],
                start=True, stop=True, tile_position=(32 * s, 0))

        A_items = []
        D_items = []
        tstate = {}
        for t in range(NUM_ENT):
            order = _ordered_pairs(t)
            tstate[t] = {
                "acc": accp.tile([H, B], BF16, tag="acc", name="acc"),
                "r": [None, None, None],
            }
            for w in range(3):
                A_items.append((t, order[2 * w], w))
                D_items.append((t, order[2 * w + 1], w))

        astate = {"idx": 0}

        def emit_a_half():
            """one A half-unit: 2 MMs + relu (+ DMA on last half)."""
            i = astate["idx"]
            if i >= 42:
                return
            astate["idx"] = i + 1
            item, half = i // 2, i % 2
            t, j, widx = A_items[item]
            if half == 0:
                tstate[t]["r"][widx] = rp.tile([H, B], BF16, tag="r",
                                               name="r")
            r = tstate[t]["r"][widx]
            hs = pslh.tile([H, HB], F32, tag="hs", name="ahs")
            for cih in range(2):
                pair_mm(t, j, hs[:, 512 * cih:512 * cih + 512],
                        HB * half + 512 * cih, 512)
            nc.scalar.activation(r[:, HB * half:HB * half + HB], hs[:],
                                 AF.Relu)
            if half == 1:
                nc.sync.dma_start(out_d[2 + widx, t, :, :], r[:])

        # D rounds: windows of 4 D-pairs x 4 quarter-columns
        nwin = (len(D_items) + 3) // 4
        PROP_AT = {2: 4, 4: 5, 5: 6}     # window -> prop target
        for k in range(nwin):
            dwin = D_items[4 * k:4 * k + 4]
            for qc in range(4):
                qslots = []
                for (t, j, didx) in dwin:
                    qslot = pslq.tile([H, QB], F32, tag="qs", name="qslot")
                    qslots.append(qslot)
                    pair_mm(t, j, qslot[:, :], QB * qc, QB)
                for (t, j, didx), qslot in zip(dwin, qslots):
                    acc = tstate[t]["acc"]
                    dst = acc[:, QB * qc:QB * qc + QB]
                    if didx == 0:
                        nc.vector.tensor_single_scalar(dst, qslot[:], 0.0,
                                                       op=ALU.max)
                    else:
                        nc.vector.scalar_tensor_tensor(
                            dst, qslot[:], 0.0, dst, op0=ALU.max, op1=ALU.add)
                    if didx == 2 and qc == 3:
                        nc.gpsimd.dma_start(out_d[1, t, :, :], acc[:])
                # ~2 A-halves per round keeps the streams 1:1
                emit_a_half()
                if (4 * k + qc) % 4 != 3:
                    emit_a_half()
            if k in PROP_AT:
                emit_prop(PROP_AT[k])
        while astate["idx"] < 42:
            emit_a_half()

    nc.compile()
    return nc


_NC_CACHE = None


def _get_nc():
    global _NC_CACHE
    if _NC_CACHE is None:
        _NC_CACHE = build()
    return _NC_CACHE


def run(ctx, w_prop, b_prop, w_rel, b_rel, trace=False):
    bf = ml_dtypes.bfloat16
    ctx = np.asarray(ctx, dtype=np.float32)
    nc = _get_nc()
    shared = build_constants(np.asarray(w_prop, np.float32),
                             np.asarray(b_prop, np.float32),
                             np.asarray(w_rel, np.float32),
                             np.asarray(b_rel, np.float32))
    in_maps = []
    for c in range(N_CORES):
        m = dict(shared)
        m["ctxb"] = np.ascontiguousarray(ctx[:, c * B:(c + 1) * B]).astype(bf)
        in_maps.append(m)
    res = run_bass_kernel_spmd(nc, in_maps, core_ids=list(range(N_CORES)),
                               trace=trace)
    shards = [np.asarray(res.results[c]["out"]).astype(np.float32)
              for c in range(N_CORES)]
    full = np.concatenate(shards, axis=3)                   # [5,7,128,16384]
    prop = full[0]
    rel = full[1] + full[2] + full[3] + full[4]
    comb = np.stack([prop, rel], axis=2)                    # [7,128,2,16384]
    out = np.transpose(comb, (3, 0, 2, 1)).reshape(B_TOTAL, NUM_ENT, 2 * H)
    return np.ascontiguousarray(out), res


def kernel(ctx, w_prop, b_prop, w_rel, b_rel):
    return run(ctx, w_prop, b_prop, w_rel, b_rel)[0]
